# revision 13
# baseline (speedup 1.0000x reference)
"""Trainium2 Bass kernel for nn_LocalAttention (sparse point-cloud attention).

Self-contained: accepts FULL unsharded inputs, shards across 8 NeuronCores
internally, returns the FULL (50000, 256) float32 output.

Distribution strategy: points (N) are sharded across the 8 cores in 6272-row
slices (128-aligned).  Neighbor indices are random over all N, so every core
builds a full neighbor-feature table [K(x) | V(x) | pos] in its own HBM
(phase A), then runs attention for its own shard (phase B), fetching neighbor
rows with indirect-DMA gathers (128 rows / 1040 B each per instruction, the
HW contract for qPoolDynamic indirect copies).

vs. the original staged version:
  - phase A: per-tile posbits DMA (392 tiny transfers) replaced by one big
    [128, G*8] load + per-tile scalar copy; K/V PSUM->row copies split
    across DVE and Act so neither engine serializes phase A; x tiles loaded
    and table rows stored two point-tiles per DMA (halves HWDGE issues).
  - Q projections for all T tiles hoisted into their own pre-gather loop
    (overlaps the table build; phase B tiles start directly with gathers).
  - phase B: b2 dropped from the logits (softmax over k is invariant to a
    per-head constant); the 16 per-neighbor AV multiplies fused into one
    4-dim broadcast multiply; the 15-op pairwise AV add tree replaced by a
    single strided tensor_reduce (f32 accumulation).

Math folds (exact, done on host):
  - softmax(l + c) == softmax(l)  -> bk and b2 drop out of the logits.
  - sum_k attn = 1                -> bv contributes bv @ Wo, folded into bo.
  - logits scale 1/sqrt(D)        -> folded into Wq, bq.
  - b1 folded into the rel-MLP via a ones-row appended to the transposed rel.
"""

import sys

import numpy as np

for _p in ("/opt/trn_rl_repo",):
    if _p not in sys.path:
        sys.path.insert(0, _p)

import ml_dtypes

import concourse.bass as bass
import concourse.tile as tile
from concourse import bacc, mybir
from concourse.bass import IndirectOffsetOnAxis
from concourse.masks import make_identity

BF16 = mybir.dt.bfloat16
F32 = mybir.dt.float32
I32 = mybir.dt.int32

N, C, H, D, K = 50000, 256, 8, 8 * 4, 16
SCALE = D**-0.5
EPS = 1e-5
NCORES = 8
P = 128
ROW = 520

nbf16 = ml_dtypes.bfloat16


def full_cfg():
    SH = 6272
    return dict(NPAD=NCORES * SH, SH=SH, T=SH // P, G=NCORES * SH // P,
                ncores=NCORES)


def host_prep(inputs, cfg):
    NPAD, SH, T, G = cfg["NPAD"], cfg["SH"], cfg["T"], cfg["G"]
    ncores = cfg["ncores"]
    n = inputs["x"].shape[0]

    x = np.asarray(inputs["x"], np.float32)
    pos = np.asarray(inputs["pos"], np.float32)
    idx = np.asarray(inputs["idx"]).astype(np.int32)
    Wq = np.asarray(inputs["Wq"], np.float32)
    bq = np.asarray(inputs["bq"], np.float32)
    Wk = np.asarray(inputs["Wk"], np.float32)
    Wv = np.asarray(inputs["Wv"], np.float32)
    Wo = np.asarray(inputs["Wo"], np.float32)
    bo = np.asarray(inputs["bo"], np.float32)
    bv = np.asarray(inputs["bv"], np.float32)
    W1 = np.asarray(inputs["W1"], np.float32)
    b1 = np.asarray(inputs["b1"], np.float32)
    W2 = np.asarray(inputs["W2"], np.float32)
    b2 = np.asarray(inputs["b2"], np.float32)
    gamma = np.asarray(inputs["gamma"], np.float32)
    beta = np.asarray(inputs["beta"], np.float32)

    xpad = np.zeros((NPAD, C), np.float32)
    xpad[:n] = x
    pospad = np.zeros((NPAD, 3), np.float32)
    pospad[:n] = pos
    idxpad = np.zeros((NPAD, K), np.int32)
    idxpad[:n] = idx

    a = xpad.reshape(G, P, 2, P)
    xtt = np.ascontiguousarray(a.transpose(0, 3, 2, 1)).astype(nbf16)
    # pair-batched layout for phase A: [g2, p, pair*2+j, m]
    xtt2 = np.ascontiguousarray(
        xtt.reshape(G // 2, 2, P, 2, P).transpose(0, 2, 1, 3, 4)
    ).reshape(G // 2, P, 4, P)

    pbits = np.zeros((NPAD, 8), np.uint16)
    pbits[:, :6] = pospad.view(np.uint16).reshape(NPAD, 6)
    pbits = np.ascontiguousarray(
        pbits.reshape(G, P, 8).transpose(1, 0, 2)).reshape(P, G * 8)
    pbits = pbits.view(nbf16)

    def wlay(W):
        return np.ascontiguousarray(
            W.reshape(2, P, C).transpose(1, 0, 2)).astype(nbf16)

    wq_l = wlay(Wq * SCALE)
    wk_l = wlay(Wk)
    wv_l = wlay(Wv)
    wo_l = wlay(Wo)
    w1aug = np.concatenate([W1, b1[None, :]], 0).astype(np.float32)
    w1big = np.zeros((4 * K, K * 64), np.float32)
    for k in range(K):
        for i in range(4):
            w1big[i * K + k, k * 64:(k + 1) * 64] = w1aug[i]
    w2big = np.zeros((P, 2 * H), np.float32)
    for kl in range(2):
        w2big[kl * 64:(kl + 1) * 64, kl * H:(kl + 1) * H] = W2
    w2big = w2big.astype(nbf16)
    bq_s = (bq * SCALE).astype(np.float32)
    bo_eff = (bv @ Wo + bo).astype(np.float32)

    shared = dict(xtt=xtt2, posbits=pbits, wq=wq_l, wk=wk_l, wv=wv_l,
                  wo=wo_l, w1b=w1big, w2=w2big, bq=bq_s, boeff=bo_eff,
                  gamma=gamma, beta=beta)

    in_maps = []
    for c in range(ncores):
        lo = c * SH
        sl = slice(lo, lo + SH)
        idxr = np.ascontiguousarray(
            idxpad[sl].reshape(T, P, K).transpose(1, 0, 2)).reshape(P, T * K)
        posr = np.ascontiguousarray(
            pospad[sl].reshape(T, P, 3).transpose(1, 0, 2)).reshape(P, T * 3)
        m = dict(shared)
        m.update(xres=np.ascontiguousarray(xpad[sl]),
                 xqt=np.ascontiguousarray(xtt[c * T:(c + 1) * T]),
                 idxr=idxr, posr=posr)
        in_maps.append(m)
    return in_maps


def build_nc(cfg):
    NPAD, SH, T, G = cfg["NPAD"], cfg["SH"], cfg["T"], cfg["G"]

    nc = bacc.Bacc(trn_type="TRN2")

    xtt = nc.dram_tensor("xtt", [G // 2, P, 4, P], BF16, kind="ExternalInput")
    posbits = nc.dram_tensor("posbits", [P, G * 8], BF16, kind="ExternalInput")
    xqt = nc.dram_tensor("xqt", [T, P, 2, P], BF16, kind="ExternalInput")
    xres = nc.dram_tensor("xres", [SH, C], F32, kind="ExternalInput")
    idxr = nc.dram_tensor("idxr", [P, T * K], I32, kind="ExternalInput")
    posr = nc.dram_tensor("posr", [P, T * 3], F32, kind="ExternalInput")
    wq = nc.dram_tensor("wq", [P, 2, C], BF16, kind="ExternalInput")
    wk = nc.dram_tensor("wk", [P, 2, C], BF16, kind="ExternalInput")
    wv = nc.dram_tensor("wv", [P, 2, C], BF16, kind="ExternalInput")
    wo = nc.dram_tensor("wo", [P, 2, C], BF16, kind="ExternalInput")
    w1b = nc.dram_tensor("w1b", [4 * K, K * 64], F32, kind="ExternalInput")
    w2 = nc.dram_tensor("w2", [P, 2 * H], BF16, kind="ExternalInput")
    bq = nc.dram_tensor("bq", [C], F32, kind="ExternalInput")
    boeff = nc.dram_tensor("boeff", [C], F32, kind="ExternalInput")
    gamma = nc.dram_tensor("gamma", [C], F32, kind="ExternalInput")
    beta = nc.dram_tensor("beta", [C], F32, kind="ExternalInput")
    y = nc.dram_tensor("y", [SH, C], F32, kind="ExternalOutput")

    table = nc.dram_tensor("table", [NPAD, ROW], BF16)

    def bcast_vec(v, cols):
        return bass.AP(tensor=v.ap().tensor, offset=0, ap=[[0, P], [1, cols]])

    with tile.TileContext(nc) as tc:
        import contextlib

        with contextlib.ExitStack() as ctx:
            consts = ctx.enter_context(tc.tile_pool(name="consts", bufs=1))

            wq_sb = consts.tile([P, 2, C], BF16)
            wk_sb = consts.tile([P, 2, C], BF16)
            wv_sb = consts.tile([P, 2, C], BF16)
            wo_sb = consts.tile([P, 2, C], BF16)
            for t_sb, t_dr in ((wq_sb, wq), (wk_sb, wk), (wv_sb, wv),
                               (wo_sb, wo)):
                nc.sync.dma_start(out=t_sb[:], in_=t_dr[:, :, :])
            w1_sb = consts.tile([4 * K, K * 64], F32)
            nc.sync.dma_start(out=w1_sb[:], in_=w1b[:, :])
            w2_sb = consts.tile([P, 2 * H], BF16)
            nc.sync.dma_start(out=w2_sb[:], in_=w2[:, :])
            idx_sb = consts.tile([P, T * K], I32)
            nc.sync.dma_start(out=idx_sb[:], in_=idxr[:, :])
            posr_sb = consts.tile([P, T * 3], F32)
            nc.sync.dma_start(out=posr_sb[:], in_=posr[:, :])
            posall = consts.tile([P, G * 8], BF16)
            nc.sync.dma_start(out=posall[:], in_=posbits[:, :])
            bq_sb = consts.tile([P, C], F32)
            nc.sync.dma_start(out=bq_sb[:], in_=bcast_vec(bq, C))
            bo_sb = consts.tile([P, C], F32)
            nc.sync.dma_start(out=bo_sb[:], in_=bcast_vec(boeff, C))
            gam_sb = consts.tile([P, C], F32)
            nc.sync.dma_start(out=gam_sb[:], in_=bcast_vec(gamma, C))
            bet_sb = consts.tile([P, C], F32)
            nc.sync.dma_start(out=bet_sb[:], in_=bcast_vec(beta, C))
            eps_sb = consts.tile([P, 1], F32)
            nc.vector.memset(eps_sb[:], EPS)
            ident = consts.tile([P, P], F32)
            make_identity(nc, ident[:])
            ident_bf = consts.tile([P, P], BF16)
            nc.vector.tensor_copy(out=ident_bf[:], in_=ident[:])

            # hoisted Q projections: all T tiles, out of the phase-B loop
            q_all = consts.tile([P, T, C], BF16)
            with tc.tile_pool(name="pq", bufs=3) as pq, \
                    tc.tile_pool(name="pqps", bufs=2, space="PSUM") as pqps:
                for t in range(T):
                    xq = pq.tile([P, 2, P], BF16, tag="xq")
                    nc.sync.dma_start(out=xq[:], in_=xqt[t, :, :, :])
                    qps = pqps.tile([P, C], F32, tag="qps")
                    for j in range(2):
                        nc.tensor.matmul(qps[:], lhsT=xq[:, j, :],
                                         rhs=wq_sb[:, j, :],
                                         start=(j == 0), stop=(j == 1))
                    nc.vector.tensor_tensor(out=q_all[:, t, :], in0=qps[:],
                                            in1=bq_sb[:],
                                            op=mybir.AluOpType.add)

            with tc.tile_pool(name="pa", bufs=4) as pa, \
                    tc.tile_pool(name="paps", bufs=2, space="PSUM") as paps:
                for g2 in range(G // 2):
                    xt = pa.tile([P, 4, P], BF16, tag="xt")
                    nc.sync.dma_start(out=xt[:], in_=xtt[g2, :, :, :])
                    row = pa.tile([P, 2, ROW], BF16, tag="row")
                    for pr in range(2):
                        g = 2 * g2 + pr
                        kps = paps.tile([P, C], F32, tag="kps")
                        vps = paps.tile([P, C], F32, tag="vps")
                        for j in range(2):
                            nc.tensor.matmul(kps[:], lhsT=xt[:, pr * 2 + j, :],
                                             rhs=wk_sb[:, j, :],
                                             start=(j == 0), stop=(j == 1))
                        for j in range(2):
                            nc.tensor.matmul(vps[:], lhsT=xt[:, pr * 2 + j, :],
                                             rhs=wv_sb[:, j, :],
                                             start=(j == 0), stop=(j == 1))
                        nc.vector.tensor_copy(out=row[:, pr, 0:C], in_=kps[:])
                        nc.scalar.copy(out=row[:, pr, C:2 * C], in_=vps[:])
                        nc.scalar.copy(out=row[:, pr, 2 * C:ROW],
                                       in_=posall[:, g * 8:(g + 1) * 8])
                    tout = bass.AP(tensor=table.ap().tensor,
                                   offset=g2 * 2 * P * ROW,
                                   ap=[[ROW, P], [P * ROW, 2], [1, ROW]])
                    nc.sync.dma_start(out=tout, in_=row[:])

            with contextlib.ExitStack() as bctx:
                pb = bctx.enter_context(tc.tile_pool(name="pb", bufs=2))
                pkv = bctx.enter_context(tc.tile_pool(name="pkv", bufs=2))
                psm = bctx.enter_context(tc.tile_pool(name="psm", bufs=2))
                ps_mm = bctx.enter_context(
                    tc.tile_pool(name="ps_mm", bufs=2, space="PSUM"))
                ps_tp = bctx.enter_context(
                    tc.tile_pool(name="ps_tp", bufs=2, space="PSUM"))
                ps_h = bctx.enter_context(
                    tc.tile_pool(name="ps_h", bufs=1, space="PSUM"))
                ps_b = bctx.enter_context(
                    tc.tile_pool(name="ps_b", bufs=2, space="PSUM"))

                for t in range(T):
                    q_sb = q_all[:, t, :]

                    kv = pkv.tile([P, K, ROW], BF16, tag="kv")
                    for k in range(K):
                        nc.gpsimd.indirect_dma_start(
                            out=kv[:, k, :], out_offset=None,
                            in_=table[:, :],
                            in_offset=IndirectOffsetOnAxis(
                                ap=idx_sb[:, t * K + k:t * K + k + 1],
                                axis=0))

                    rel = pb.tile([P, 4, K], F32, tag="rel")
                    nc.vector.memset(rel[:, 3, :], 1.0)
                    pos_nb = kv[:, :, 2 * C:2 * C + 6].bitcast(F32)
                    pnb = bass.AP(tensor=pos_nb.tensor, offset=pos_nb.offset,
                                  ap=[pos_nb.ap[0], [1, 3],
                                      [ROW // 2, K]])
                    psl = posr_sb[:, t * 3:(t + 1) * 3]
                    pbr = bass.AP(tensor=psl.tensor, offset=psl.offset,
                                  ap=[psl.ap[0], [1, 3], [0, K]])
                    nc.vector.tensor_tensor(out=rel[:, 0:3, :], in0=pnb,
                                            in1=pbr,
                                            op=mybir.AluOpType.subtract)

                    relT_ps = ps_tp.tile([64, P], F32, tag="tp")
                    rel2d = rel[:].rearrange("p a k -> p (a k)")
                    nc.tensor.transpose(out=relT_ps[:], in_=rel2d,
                                        identity=ident[:])
                    relT = pb.tile([64, P], F32, tag="relT")
                    nc.scalar.copy(out=relT[:], in_=relT_ps[:])
                    hid_ps = ps_h.tile([P, K * 64], F32, tag="hid")
                    for j in range(2):
                        nc.tensor.matmul(hid_ps[:, j * 512:(j + 1) * 512],
                                         lhsT=relT[:],
                                         rhs=w1_sb[:, j * 512:(j + 1) * 512],
                                         start=True, stop=True)
                    hid_sb = pb.tile([P, K * 64], BF16, tag="hid_sb")
                    nc.scalar.activation(out=hid_sb[:], in_=hid_ps[:],
                                         func=mybir.ActivationFunctionType.Gelu)
                    hidT_ps = ps_tp.tile([P, 8, P], BF16, tag="tp")
                    for b in range(8):
                        nc.tensor.transpose(out=hidT_ps[:, b, :],
                                            in_=hid_sb[:, b * P:(b + 1) * P],
                                            identity=ident_bf[:])
                    hidT = pb.tile([P, 8, P], BF16, tag="hidT")
                    nc.scalar.copy(out=hidT[:], in_=hidT_ps[:])
                    bias_ps = ps_b.tile([P, K, H], F32, tag="bias")
                    for b in range(8):
                        nc.tensor.matmul(
                            bias_ps[:, 2 * b:2 * b + 2, :].rearrange(
                                "p a h -> p (a h)"),
                            lhsT=hidT[:, b, :], rhs=w2_sb[:],
                            start=True, stop=True)

                    prod = pkv.tile([P, K, H, D], BF16, tag="prod")
                    kf = bass.AP(tensor=kv.tensor, offset=kv[:].offset,
                                 ap=[kv[:].ap[0], [ROW, K], [D, H], [1, D]])
                    qb = bass.AP(tensor=q_sb.tensor, offset=q_sb.offset,
                                 ap=[q_sb.ap[0], [0, K], [D, H], [1, D]])
                    nc.vector.tensor_tensor(out=prod[:], in0=kf, in1=qb,
                                            op=mybir.AluOpType.mult)
                    logits = psm.tile([P, K, H], F32, tag="log")
                    nc.vector.tensor_reduce(out=logits[:], in_=prod[:],
                                            axis=mybir.AxisListType.X,
                                            op=mybir.AluOpType.add)
                    nc.vector.tensor_tensor(out=logits[:], in0=logits[:],
                                            in1=bias_ps[:],
                                            op=mybir.AluOpType.add)

                    pex = psm.tile([P, K, H], F32, tag="pex")
                    nc.scalar.activation(out=pex[:], in_=logits[:],
                                         func=mybir.ActivationFunctionType.Exp)
                    ssum = psm.tile([P, H], F32, tag="ssum")
                    pex_hk = bass.AP(tensor=pex.tensor, offset=pex[:].offset,
                                     ap=[pex[:].ap[0], [1, H], [H, K]])
                    nc.vector.tensor_reduce(out=ssum[:], in_=pex_hk,
                                            axis=mybir.AxisListType.X,
                                            op=mybir.AluOpType.add)
                    rinv = psm.tile([P, H], F32, tag="rinv")
                    nc.vector.reciprocal(out=rinv[:], in_=ssum[:])
                    attn = pb.tile([P, K, H], BF16, tag="attn")
                    rib = bass.AP(tensor=rinv.tensor, offset=rinv[:].offset,
                                  ap=[rinv[:].ap[0], [0, K], [1, H]])
                    nc.vector.tensor_tensor(out=attn[:], in0=pex[:], in1=rib,
                                            op=mybir.AluOpType.mult)

                    av = pkv.tile([P, K, C], BF16, tag="av")
                    vf = bass.AP(tensor=kv.tensor, offset=kv[:].offset + C,
                                 ap=[kv[:].ap[0], [ROW, K], [D, H], [1, D]])
                    ab = bass.AP(tensor=attn.tensor, offset=attn[:].offset,
                                 ap=[attn[:].ap[0], [H, K], [1, H], [0, D]])
                    nc.vector.tensor_tensor(
                        out=av[:].rearrange("p s (h d) -> p s h d", h=H),
                        in0=vf, in1=ab, op=mybir.AluOpType.mult)
                    avs = psm.tile([P, C], F32, tag="avs")
                    avr = bass.AP(tensor=av.tensor, offset=av[:].offset,
                                  ap=[av[:].ap[0], [1, C], [C, K]])
                    nc.vector.tensor_reduce(out=avs[:], in_=avr,
                                            axis=mybir.AxisListType.X,
                                            op=mybir.AluOpType.add)
                    avsb = pb.tile([P, C], BF16, tag="avsb")
                    nc.scalar.copy(out=avsb[:], in_=avs[:])

                    oT_ps = ps_tp.tile([P, 2, P], BF16, tag="tp")
                    for j in range(2):
                        nc.tensor.transpose(out=oT_ps[:, j, :],
                                            in_=avsb[:, j * P:(j + 1) * P],
                                            identity=ident_bf[:])
                    oT = pb.tile([P, 2, P], BF16, tag="oT")
                    nc.scalar.copy(out=oT[:], in_=oT_ps[:])
                    ops = ps_mm.tile([P, C], F32, tag="mm")
                    for j in range(2):
                        nc.tensor.matmul(ops[:], lhsT=oT[:, j, :],
                                         rhs=wo_sb[:, j, :],
                                         start=(j == 0), stop=(j == 1))

                    xr = pb.tile([P, C], F32, tag="xr")
                    nc.sync.dma_start(out=xr[:],
                                      in_=xres[t * P:(t + 1) * P, :])
                    y0 = pb.tile([P, C], F32, tag="y0")
                    nc.vector.tensor_tensor(out=y0[:], in0=ops[:],
                                            in1=bo_sb[:],
                                            op=mybir.AluOpType.add)
                    nc.vector.tensor_tensor(out=y0[:], in0=y0[:], in1=xr[:],
                                            op=mybir.AluOpType.add)
                    bst = psm.tile([P, 6], F32, tag="bst")
                    nc.vector.bn_stats(out=bst[:], in_=y0[:])
                    mv = psm.tile([P, 2], F32, tag="mv")
                    nc.vector.bn_aggr(out=mv[:], in_=bst[:])
                    std = psm.tile([P, 1], F32, tag="std")
                    nc.scalar.activation(out=std[:], in_=mv[:, 1:2],
                                         func=mybir.ActivationFunctionType.Sqrt,
                                         bias=eps_sb[:])
                    rstd = psm.tile([P, 1], F32, tag="rstd")
                    nc.vector.reciprocal(out=rstd[:], in_=std[:])
                    nc.vector.tensor_scalar(out=y0[:], in0=y0[:],
                                            scalar1=mv[:, 0:1],
                                            scalar2=rstd[:],
                                            op0=mybir.AluOpType.subtract,
                                            op1=mybir.AluOpType.mult)
                    yt = pb.tile([P, C], F32, tag="yt")
                    nc.vector.tensor_tensor(out=yt[:], in0=y0[:],
                                            in1=gam_sb[:],
                                            op=mybir.AluOpType.mult)
                    nc.vector.tensor_tensor(out=yt[:], in0=yt[:],
                                            in1=bet_sb[:],
                                            op=mybir.AluOpType.add)
                    nc.sync.dma_start(out=y[t * P:(t + 1) * P, :], in_=yt[:])

    nc.compile()
    return nc


# ------------------------------------------------------------------ driver

_NC_CACHE = {}
RUN_KWARGS = {}
LAST_RESULT = None


def _get_nc(cfg_key, cfg):
    if cfg_key not in _NC_CACHE:
        _NC_CACHE[cfg_key] = build_nc(cfg)
    return _NC_CACHE[cfg_key]


def kernel(**inputs):
    global LAST_RESULT
    from concourse.bass_utils import run_bass_kernel_spmd

    cfg = full_cfg()
    in_maps = host_prep(inputs, cfg)
    nc = _get_nc("full", cfg)
    res = run_bass_kernel_spmd(nc, in_maps, core_ids=list(range(NCORES)),
                               **RUN_KWARGS)
    LAST_RESULT = res
    y = np.concatenate([res.results[c]["y"] for c in range(NCORES)], 0)
    return np.ascontiguousarray(y[:N])


if __name__ == "__main__":
    pass
